# revision 45
# baseline (speedup 1.0000x reference)
"""KL-divergence loss kernel (C51 categorical projection + batchmean KL) for TRN2.

Math: the reference projects `anchor` through a C51 projection whose skew is a
compile-time scalar, so the projection collapses to a constant linear map:

    t[:, 0]  = 0
    t[:, 1]  = 0.75*a[:, 0]
    t[:, j]  = 0.75*a[:, j-1] + 0.25*a[:, j-2]          (2 <= j <= 49)
    t[:, 50] = 0.25*a[:, 48] + a[:, 49] + a[:, 50]

and the loss is sum(t * (log t - log(f + 1e-16))) / B  (terms with t==0 are 0).

Kernel strategy (pure data parallel over 8 cores, batch-sharded):
  s = 4t built with one wide fused scalar_tensor_tensor (s_j = 3*a_{j-1} + a_{j-2})
  on VectorE; the three boundary-column fixups run on the otherwise-idle GpSimd.
  lt = Ln(s + 1e-35)           [ScalarE, fused affine] -> bf16
  lf = Ln(4f + 4e-16)          [ScalarE, fused affine] -> bf16 (folds 1/4 of s)
  d  = lt - lf  in place       [VectorE, all-bf16 for the 2x DVE mode]
  sum s*d via TensorE: accumulate lhsT=s_blk, rhs=d_blk matmuls into one
  [128,128] PSUM tile; only the diagonal is meaningful. Host sums diagonals
  of the 8 per-core results and scales by 0.25/B.

Tiles are non-uniform (small first tile for a fast pipeline fill, small last
tile for a short drain; big middle tiles for low instruction overhead).
"""

import os
import numpy as np

B_TOTAL = 524288
ATOMS = 51
N_CORES = 8
ROWS_PER_CORE = B_TOTAL // N_CORES  # 65536 = 128 * 512
P = 128
R_TILES = [48, 80, 80, 80, 80, 80, 32, 32]  # rows/partition per tile; sum=512
R_MAX = max(R_TILES)
MM_BLOCK = 128

_BUILT = None
_LAST_RESULTS = None


def _build():
    from contextlib import ExitStack

    import concourse.bacc as bacc
    import concourse.tile as tile
    from concourse import mybir

    assert sum(R_TILES) * P == ROWS_PER_CORE

    nc = bacc.Bacc("TRN2", num_devices=N_CORES)

    a_dram = nc.dram_tensor(
        "anchor", [ROWS_PER_CORE, ATOMS], mybir.dt.float32, kind="ExternalInput"
    )
    f_dram = nc.dram_tensor(
        "feature", [ROWS_PER_CORE, ATOMS], mybir.dt.float32, kind="ExternalInput"
    )
    out_dram = nc.dram_tensor(
        "out", [P, MM_BLOCK], mybir.dt.float32, kind="ExternalOutput"
    )

    mult = mybir.AluOpType.mult
    add = mybir.AluOpType.add

    # Per-tile matmul block decomposition and global matmul count.
    tile_blocks = []
    for r in R_TILES:
        cols = r * ATOMS
        n_full, tail = divmod(cols, MM_BLOCK)
        blocks = [(b * MM_BLOCK, MM_BLOCK) for b in range(n_full)]
        if tail:
            blocks.append((n_full * MM_BLOCK, tail))
        tile_blocks.append(blocks)
    total_mms = sum(len(b) for b in tile_blocks)

    with tile.TileContext(nc) as tc:
        with ExitStack() as ctx:
            a_pool = ctx.enter_context(tc.tile_pool(name="a", bufs=3))
            f_pool = ctx.enter_context(tc.tile_pool(name="f", bufs=3))
            s_pool = ctx.enter_context(tc.tile_pool(name="s", bufs=4))
            lt_pool = ctx.enter_context(tc.tile_pool(name="lt", bufs=4))
            lf_pool = ctx.enter_context(tc.tile_pool(name="lf", bufs=4))
            tmp_pool = ctx.enter_context(tc.tile_pool(name="tmp", bufs=2))
            out_pool = ctx.enter_context(tc.tile_pool(name="outp", bufs=1))
            psum_pool = ctx.enter_context(
                tc.tile_pool(name="acc", bufs=1, space="PSUM")
            )

            acc = psum_pool.tile([P, MM_BLOCK], mybir.dt.float32)

            eps_s = out_pool.tile([P, 1], mybir.dt.float32, tag="eps_s")
            eps_f = out_pool.tile([P, 1], mybir.dt.float32, tag="eps_f")
            warm = out_pool.tile([P, 1], mybir.dt.float32, tag="warm")
            nc.gpsimd.memset(eps_s[:], 1e-35)
            nc.gpsimd.memset(eps_f[:], 4e-16)
            # Load the Ln activation table while the first DMAs are in flight.
            nc.scalar.activation(
                out=warm[:],
                in_=eps_s[:],
                func=mybir.ActivationFunctionType.Ln,
                bias=eps_f[:],
                scale=1.0,
            )

            def consume(prev):
                # One-tile-delayed tail: d = lt - lf in place (all-bf16 ->
                # 2x DVE mode), then the trace-trick matmuls.
                nonlocal mm
                s_sb, lt_sb, lf_sb, cols, blocks = prev
                nc.vector.tensor_sub(
                    lt_sb[:, :cols], lt_sb[:, :cols], lf_sb[:, :cols]
                )
                for c0, w in blocks:
                    nc.tensor.matmul(
                        acc[0:w, 0:w],
                        s_sb[:, c0 : c0 + w],
                        lt_sb[:, c0 : c0 + w],
                        start=(mm == 0),
                        stop=(mm == total_mms - 1),
                    )
                    mm += 1

            mm = 0
            row0 = 0
            pending = []
            for i, r in enumerate(R_TILES):
                cols = r * ATOMS
                a_view = a_dram.ap()[row0 : row0 + P * r, :].rearrange(
                    "(p q) m -> p (q m)", p=P
                )
                f_view = f_dram.ap()[row0 : row0 + P * r, :].rearrange(
                    "(p q) m -> p (q m)", p=P
                )
                row0 += P * r

                a_sb = a_pool.tile([P, R_MAX * ATOMS], mybir.dt.float32)
                f_sb = f_pool.tile([P, R_MAX * ATOMS], mybir.dt.float32)
                nc.sync.dma_start(out=a_sb[:, :cols], in_=a_view)
                # The last big tile's f is split in half so its lf can
                # start when the first half lands, pulling the ScalarE
                # tail chain earlier.
                if i == len(R_TILES) - 3:
                    h = (r // 2) * ATOMS
                    f_halves = [(0, h), (h, cols)]
                else:
                    f_halves = [(0, cols)]
                # f rides the Activation engine's HWDGE queue so the two
                # input streams use two hardware DMA queues in parallel.
                # The last two tiles' f go on the sync queue instead: the
                # a-stream is done by then, and a trigger stuck behind a
                # tail ACT on the Scalar queue would delay the drain.
                for h0, h1 in f_halves:
                    if i >= len(R_TILES) - 2:
                        nc.sync.dma_start(out=f_sb[:, h0:h1], in_=f_view[:, h0:h1])
                    else:
                        nc.scalar.dma_start(out=f_sb[:, h0:h1], in_=f_view[:, h0:h1])

                s_sb = s_pool.tile([P, R_MAX * ATOMS], mybir.dt.bfloat16)
                lt_sb = lt_pool.tile([P, R_MAX * ATOMS], mybir.dt.bfloat16)
                lf_sb = lf_pool.tile([P, R_MAX * ATOMS], mybir.dt.bfloat16)
                tmp = tmp_pool.tile([P, R_MAX], mybir.dt.float32)

                a3 = a_sb[:, :cols].rearrange("p (q m) -> p q m", m=ATOMS)
                s3 = s_sb[:, :cols].rearrange("p (q m) -> p q m", m=ATOMS)
                t1 = tmp[:, 0:r]

                # Boundary columns first (small, keeps the queue ahead of
                # the wide stt that lt waits on): s1 = 3*a0,
                # s50 = a48 + 4*a49 + 4*a50; s0 = 0. All on VectorE so s
                # has a single producer engine (fewer cross-engine waits).
                nc.vector.memset(s3[:, :, 0], 0.0)
                nc.vector.tensor_scalar_mul(s3[:, :, 1], a3[:, :, 0], 3.0)
                nc.vector.scalar_tensor_tensor(
                    out=t1,
                    in0=a3[:, :, 49],
                    scalar=4.0,
                    in1=a3[:, :, 48],
                    op0=mult,
                    op1=add,
                )
                nc.vector.scalar_tensor_tensor(
                    out=s3[:, :, 50],
                    in0=a3[:, :, 50],
                    scalar=4.0,
                    in1=t1,
                    op0=mult,
                    op1=add,
                )
                # s_j = 3*a_{j-1} + a_{j-2} for j in 2..49  (VectorE, wide)
                nc.vector.scalar_tensor_tensor(
                    out=s3[:, :, 2:50],
                    in0=a3[:, :, 1:49],
                    scalar=3.0,
                    in1=a3[:, :, 0:48],
                    op0=mult,
                    op1=add,
                )

                # lf = Ln(4f + 4e-16); lt = Ln(s + 1e-35)  (ScalarE, bf16
                # out). lf first: it depends only on the f DMA, so ScalarE
                # can run it while VectorE is still building s.
                for h0, h1 in f_halves:
                    nc.scalar.activation(
                        out=lf_sb[:, h0:h1],
                        in_=f_sb[:, h0:h1],
                        func=mybir.ActivationFunctionType.Ln,
                        bias=eps_f[:],
                        scale=4.0,
                    )
                nc.scalar.activation(
                    out=lt_sb[:, :cols],
                    in_=s_sb[:, :cols],
                    func=mybir.ActivationFunctionType.Ln,
                    bias=eps_s[:],
                    scale=1.0,
                )
                pending.append((s_sb, lt_sb, lf_sb, cols, tile_blocks[i]))
                if len(pending) > 2:
                    consume(pending.pop(0))

            for p in pending:
                consume(p)

            out_sb = out_pool.tile([P, MM_BLOCK], mybir.dt.float32, tag="out_sb")
            nc.vector.tensor_copy(out_sb[:], acc[:])
            nc.sync.dma_start(out=out_dram.ap(), in_=out_sb[:])

    nc.compile()
    return nc


def kernel(anchor: np.ndarray, feature: np.ndarray) -> np.ndarray:
    global _BUILT, _LAST_RESULTS
    from concourse import bass_utils

    if _BUILT is None:
        _BUILT = _build()
    nc = _BUILT

    anchor = np.ascontiguousarray(anchor, dtype=np.float32)
    feature = np.ascontiguousarray(feature, dtype=np.float32)

    in_maps = []
    for c in range(N_CORES):
        lo, hi = c * ROWS_PER_CORE, (c + 1) * ROWS_PER_CORE
        in_maps.append({"anchor": anchor[lo:hi], "feature": feature[lo:hi]})

    res = bass_utils.run_bass_kernel_spmd(
        nc,
        in_maps,
        core_ids=list(range(N_CORES)),
        trace=bool(os.environ.get("BASS_TRACE")),
    )
    _LAST_RESULTS = res

    total = 0.0
    for c in range(N_CORES):
        total += np.trace(res.results[c]["out"].astype(np.float64))
    val = 0.25 * total / B_TOTAL
    return np.float32(val)


# revision 48
# speedup vs baseline: 1.1581x; 1.1581x over previous
"""KL-divergence loss kernel (C51 categorical projection + batchmean KL) for TRN2.

Math: the reference projects `anchor` through a C51 projection whose skew is a
compile-time scalar, so the projection collapses to a constant linear map:

    t[:, 0]  = 0
    t[:, 1]  = 0.75*a[:, 0]
    t[:, j]  = 0.75*a[:, j-1] + 0.25*a[:, j-2]          (2 <= j <= 49)
    t[:, 50] = 0.25*a[:, 48] + a[:, 49] + a[:, 50]

and the loss is sum(t * (log t - log(f + 1e-16))) / B  (terms with t==0 are 0).

Kernel strategy (pure data parallel over 8 cores, batch-sharded):
  a streams on the sync-engine DMA queue, f on the scalar-engine queue —
  the two hardware queues run in parallel and together saturate the
  per-core HBM bandwidth (~400 B/ns combined).
  s = 4t built with one wide fused scalar_tensor_tensor (s_j = 3*a_{j-1} +
  a_{j-2}) plus boundary-column fixups, all on VectorE (single producer).
  lf = Ln(4f + 4e-16)          [ScalarE, fused affine] -> bf16 (folds 1/4 of s)
  lt = Ln(s + 1e-35)           [ScalarE, fused affine] -> bf16
  d  = lt - lf  in place       [VectorE, all-bf16 for the 2x DVE mode]
  sum s*d via TensorE: accumulate lhsT=s_blk, rhs=d_blk matmuls into one
  [128,128] PSUM tile; only the diagonal is meaningful. Host sums diagonals
  of the 8 per-core results and scales by 0.25/B.

The sub+matmul stage trails the producer stage by TWO tiles: the Tile
scheduler orders each engine's queue from a simulated timeline whose DMA
model runs slow, and with a shorter skew it parks sub_i directly in front
of stt_{i+1} in the Vector queue, stalling the pipeline on real hardware.
Tiles are non-uniform (small first tile for a fast pipeline fill, small
last tiles for a short drain; big middle tiles for low overhead). This
exact tile schedule is a measured local optimum — small perturbations
made the emitted schedule significantly worse.
"""

import os
import numpy as np

B_TOTAL = 524288
ATOMS = 51
N_CORES = 8
ROWS_PER_CORE = B_TOTAL // N_CORES  # 65536 = 128 * 512
P = 128
R_TILES = [48, 80, 80, 80, 80, 80, 32, 32]  # rows/partition per tile; sum=512
R_MAX = max(R_TILES)
MM_BLOCK = 128

_BUILT = None
_LAST_RESULTS = None


def _build():
    from contextlib import ExitStack

    import concourse.bacc as bacc
    import concourse.tile as tile
    from concourse import mybir

    assert sum(R_TILES) * P == ROWS_PER_CORE

    nc = bacc.Bacc("TRN2", num_devices=N_CORES)

    a_dram = nc.dram_tensor(
        "anchor", [ROWS_PER_CORE, ATOMS], mybir.dt.float32, kind="ExternalInput"
    )
    f_dram = nc.dram_tensor(
        "feature", [ROWS_PER_CORE, ATOMS], mybir.dt.float32, kind="ExternalInput"
    )
    out_dram = nc.dram_tensor(
        "out", [P, MM_BLOCK], mybir.dt.float32, kind="ExternalOutput"
    )

    mult = mybir.AluOpType.mult
    add = mybir.AluOpType.add

    # Per-tile matmul block decomposition and global matmul count.
    tile_blocks = []
    for r in R_TILES:
        cols = r * ATOMS
        n_full, tail = divmod(cols, MM_BLOCK)
        blocks = [(b * MM_BLOCK, MM_BLOCK) for b in range(n_full)]
        if tail:
            blocks.append((n_full * MM_BLOCK, tail))
        tile_blocks.append(blocks)
    total_mms = sum(len(b) for b in tile_blocks)

    with tile.TileContext(nc) as tc:
        with ExitStack() as ctx:
            a_pool = ctx.enter_context(tc.tile_pool(name="a", bufs=3))
            f_pool = ctx.enter_context(tc.tile_pool(name="f", bufs=3))
            s_pool = ctx.enter_context(tc.tile_pool(name="s", bufs=4))
            lt_pool = ctx.enter_context(tc.tile_pool(name="lt", bufs=4))
            lf_pool = ctx.enter_context(tc.tile_pool(name="lf", bufs=4))
            tmp_pool = ctx.enter_context(tc.tile_pool(name="tmp", bufs=2))
            out_pool = ctx.enter_context(tc.tile_pool(name="outp", bufs=1))
            psum_pool = ctx.enter_context(
                tc.tile_pool(name="acc", bufs=1, space="PSUM")
            )

            acc = psum_pool.tile([P, MM_BLOCK], mybir.dt.float32)

            eps_s = out_pool.tile([P, 1], mybir.dt.float32, tag="eps_s")
            eps_f = out_pool.tile([P, 1], mybir.dt.float32, tag="eps_f")
            warm = out_pool.tile([P, 1], mybir.dt.float32, tag="warm")
            nc.gpsimd.memset(eps_s[:], 1e-35)
            nc.gpsimd.memset(eps_f[:], 4e-16)
            # Load the Ln activation table while the first DMAs are in flight.
            nc.scalar.activation(
                out=warm[:],
                in_=eps_s[:],
                func=mybir.ActivationFunctionType.Ln,
                bias=eps_f[:],
                scale=1.0,
            )

            def consume(prev):
                # One-tile-delayed tail: d = lt - lf in place (all-bf16 ->
                # 2x DVE mode), then the trace-trick matmuls.
                nonlocal mm
                s_sb, lt_sb, lf_sb, cols, blocks = prev
                nc.vector.tensor_sub(
                    lt_sb[:, :cols], lt_sb[:, :cols], lf_sb[:, :cols]
                )
                for c0, w in blocks:
                    nc.tensor.matmul(
                        acc[0:w, 0:w],
                        s_sb[:, c0 : c0 + w],
                        lt_sb[:, c0 : c0 + w],
                        start=(mm == 0),
                        stop=(mm == total_mms - 1),
                    )
                    mm += 1

            mm = 0
            row0 = 0
            pending = []
            for i, r in enumerate(R_TILES):
                cols = r * ATOMS
                a_view = a_dram.ap()[row0 : row0 + P * r, :].rearrange(
                    "(p q) m -> p (q m)", p=P
                )
                f_view = f_dram.ap()[row0 : row0 + P * r, :].rearrange(
                    "(p q) m -> p (q m)", p=P
                )
                row0 += P * r

                a_sb = a_pool.tile([P, R_MAX * ATOMS], mybir.dt.float32)
                f_sb = f_pool.tile([P, R_MAX * ATOMS], mybir.dt.float32)
                nc.sync.dma_start(out=a_sb[:, :cols], in_=a_view)
                # f rides the Activation engine's HWDGE queue so the two
                # input streams use two hardware DMA queues in parallel.
                # The last two tiles' f go on the sync queue instead: the
                # a-stream is done by then, and a trigger stuck behind a
                # tail ACT on the Scalar queue would delay the drain.
                if i >= len(R_TILES) - 2:
                    nc.sync.dma_start(out=f_sb[:, :cols], in_=f_view)
                else:
                    nc.scalar.dma_start(out=f_sb[:, :cols], in_=f_view)

                s_sb = s_pool.tile([P, R_MAX * ATOMS], mybir.dt.bfloat16)
                lt_sb = lt_pool.tile([P, R_MAX * ATOMS], mybir.dt.bfloat16)
                lf_sb = lf_pool.tile([P, R_MAX * ATOMS], mybir.dt.bfloat16)
                tmp = tmp_pool.tile([P, R_MAX], mybir.dt.float32)

                a3 = a_sb[:, :cols].rearrange("p (q m) -> p q m", m=ATOMS)
                s3 = s_sb[:, :cols].rearrange("p (q m) -> p q m", m=ATOMS)
                t1 = tmp[:, 0:r]

                # Boundary columns first (small, keeps the queue ahead of
                # the wide stt that lt waits on): s1 = 3*a0,
                # s50 = a48 + 4*a49 + 4*a50; s0 = 0. All on VectorE so s
                # has a single producer engine (fewer cross-engine waits).
                nc.vector.memset(s3[:, :, 0], 0.0)
                nc.vector.tensor_scalar_mul(s3[:, :, 1], a3[:, :, 0], 3.0)
                nc.vector.scalar_tensor_tensor(
                    out=t1,
                    in0=a3[:, :, 49],
                    scalar=4.0,
                    in1=a3[:, :, 48],
                    op0=mult,
                    op1=add,
                )
                nc.vector.scalar_tensor_tensor(
                    out=s3[:, :, 50],
                    in0=a3[:, :, 50],
                    scalar=4.0,
                    in1=t1,
                    op0=mult,
                    op1=add,
                )
                # s_j = 3*a_{j-1} + a_{j-2} for j in 2..49  (VectorE, wide)
                nc.vector.scalar_tensor_tensor(
                    out=s3[:, :, 2:50],
                    in0=a3[:, :, 1:49],
                    scalar=3.0,
                    in1=a3[:, :, 0:48],
                    op0=mult,
                    op1=add,
                )

                # lf = Ln(4f + 4e-16); lt = Ln(s + 1e-35)  (ScalarE, bf16
                # out). lf first: it depends only on the f DMA, so ScalarE
                # can run it while VectorE is still building s.
                nc.scalar.activation(
                    out=lf_sb[:, :cols],
                    in_=f_sb[:, :cols],
                    func=mybir.ActivationFunctionType.Ln,
                    bias=eps_f[:],
                    scale=4.0,
                )
                nc.scalar.activation(
                    out=lt_sb[:, :cols],
                    in_=s_sb[:, :cols],
                    func=mybir.ActivationFunctionType.Ln,
                    bias=eps_s[:],
                    scale=1.0,
                )
                pending.append((s_sb, lt_sb, lf_sb, cols, tile_blocks[i]))
                if len(pending) > 2:
                    consume(pending.pop(0))

            for p in pending:
                consume(p)

            out_sb = out_pool.tile([P, MM_BLOCK], mybir.dt.float32, tag="out_sb")
            nc.vector.tensor_copy(out_sb[:], acc[:])
            nc.sync.dma_start(out=out_dram.ap(), in_=out_sb[:])

    nc.compile()
    return nc


def kernel(anchor: np.ndarray, feature: np.ndarray) -> np.ndarray:
    global _BUILT, _LAST_RESULTS
    from concourse import bass_utils

    if _BUILT is None:
        _BUILT = _build()
    nc = _BUILT

    anchor = np.ascontiguousarray(anchor, dtype=np.float32)
    feature = np.ascontiguousarray(feature, dtype=np.float32)

    in_maps = []
    for c in range(N_CORES):
        lo, hi = c * ROWS_PER_CORE, (c + 1) * ROWS_PER_CORE
        in_maps.append({"anchor": anchor[lo:hi], "feature": feature[lo:hi]})

    res = bass_utils.run_bass_kernel_spmd(
        nc,
        in_maps,
        core_ids=list(range(N_CORES)),
        trace=bool(os.environ.get("BASS_TRACE")),
    )
    _LAST_RESULTS = res

    total = 0.0
    for c in range(N_CORES):
        total += np.trace(res.results[c]["out"].astype(np.float64))
    val = 0.25 * total / B_TOTAL
    return np.float32(val)


# revision 49
# speedup vs baseline: 1.1880x; 1.0258x over previous
"""KL-divergence loss kernel (C51 categorical projection + batchmean KL) for TRN2.

Math: the reference projects `anchor` through a C51 projection whose skew is a
compile-time scalar, so the projection collapses to a constant linear map:

    t[:, 0]  = 0
    t[:, 1]  = 0.75*a[:, 0]
    t[:, j]  = 0.75*a[:, j-1] + 0.25*a[:, j-2]          (2 <= j <= 49)
    t[:, 50] = 0.25*a[:, 48] + a[:, 49] + a[:, 50]

and the loss is sum(t * (log t - log(f + 1e-16))) / B  (terms with t==0 are 0).

Kernel strategy (pure data parallel over 8 cores, batch-sharded):
  a streams on the sync-engine DMA queue, f on the scalar-engine queue —
  the two hardware queues run in parallel and together saturate the
  per-core HBM bandwidth (~400 B/ns combined).
  s = 4t built with one wide fused scalar_tensor_tensor (s_j = 3*a_{j-1} +
  a_{j-2}) plus boundary-column fixups, all on VectorE (single producer).
  lf = Ln(4f + 4e-16)          [ScalarE, fused affine] -> bf16 (folds 1/4 of s)
  lt = Ln(s + 1e-35)           [ScalarE, fused affine] -> bf16
  d  = lt - lf  in place       [VectorE, all-bf16 for the 2x DVE mode]
  sum s*d via TensorE: accumulate lhsT=s_blk, rhs=d_blk matmuls into one
  [128,128] PSUM tile; only the diagonal is meaningful. Host sums diagonals
  of the 8 per-core results and scales by 0.25/B.

The sub+matmul stage trails the producer stage by TWO tiles: the Tile
scheduler orders each engine's queue from a simulated timeline whose DMA
model runs slow, and with a shorter skew it parks sub_i directly in front
of stt_{i+1} in the Vector queue, stalling the pipeline on real hardware.
Tiles are non-uniform (small first tile for a fast pipeline fill, small
last tiles for a short drain; big middle tiles for low overhead). This
exact tile schedule is a measured local optimum — small perturbations
made the emitted schedule significantly worse.
"""

import os
import numpy as np

B_TOTAL = 524288
ATOMS = 51
N_CORES = 8
ROWS_PER_CORE = B_TOTAL // N_CORES  # 65536 = 128 * 512
P = 128
R_TILES = [48, 80, 80, 80, 80, 80, 32, 32]  # rows/partition per tile; sum=512
R_MAX = max(R_TILES)
MM_BLOCK = 128

_BUILT = None
_LAST_RESULTS = None


def _build():
    from contextlib import ExitStack

    import concourse.bacc as bacc
    import concourse.tile as tile
    from concourse import mybir

    assert sum(R_TILES) * P == ROWS_PER_CORE

    nc = bacc.Bacc("TRN2", num_devices=N_CORES)

    a_dram = nc.dram_tensor(
        "anchor", [ROWS_PER_CORE, ATOMS], mybir.dt.float32, kind="ExternalInput"
    )
    f_dram = nc.dram_tensor(
        "feature", [ROWS_PER_CORE, ATOMS], mybir.dt.float32, kind="ExternalInput"
    )
    out_dram = nc.dram_tensor(
        "out", [P, MM_BLOCK], mybir.dt.float32, kind="ExternalOutput"
    )

    mult = mybir.AluOpType.mult
    add = mybir.AluOpType.add

    # Per-tile matmul block decomposition and global matmul count.
    tile_blocks = []
    for r in R_TILES:
        cols = r * ATOMS
        n_full, tail = divmod(cols, MM_BLOCK)
        blocks = [(b * MM_BLOCK, MM_BLOCK) for b in range(n_full)]
        if tail:
            blocks.append((n_full * MM_BLOCK, tail))
        tile_blocks.append(blocks)
    total_mms = sum(len(b) for b in tile_blocks)

    with tile.TileContext(nc) as tc:
        with ExitStack() as ctx:
            a_pool = ctx.enter_context(tc.tile_pool(name="a", bufs=3))
            f_pool = ctx.enter_context(tc.tile_pool(name="f", bufs=3))
            s_pool = ctx.enter_context(tc.tile_pool(name="s", bufs=4))
            lt_pool = ctx.enter_context(tc.tile_pool(name="lt", bufs=4))
            lf_pool = ctx.enter_context(tc.tile_pool(name="lf", bufs=4))
            tmp_pool = ctx.enter_context(tc.tile_pool(name="tmp", bufs=2))
            out_pool = ctx.enter_context(tc.tile_pool(name="outp", bufs=1))
            psum_pool = ctx.enter_context(
                tc.tile_pool(name="acc", bufs=1, space="PSUM")
            )

            acc = psum_pool.tile([P, MM_BLOCK], mybir.dt.float32)

            eps_s = out_pool.tile([P, 1], mybir.dt.float32, tag="eps_s")
            eps_f = out_pool.tile([P, 1], mybir.dt.float32, tag="eps_f")
            warm = out_pool.tile([P, 1], mybir.dt.float32, tag="warm")
            nc.gpsimd.memset(eps_s[:], 1e-35)
            nc.gpsimd.memset(eps_f[:], 4e-16)
            # Load the Ln activation table while the first DMAs are in flight.
            nc.scalar.activation(
                out=warm[:],
                in_=eps_s[:],
                func=mybir.ActivationFunctionType.Ln,
                bias=eps_f[:],
                scale=1.0,
            )

            def consume(prev):
                # One-tile-delayed tail: d = lt - lf in place (all-bf16 ->
                # 2x DVE mode), then the trace-trick matmuls.
                nonlocal mm
                s_sb, lt_sb, lf_sb, cols, blocks = prev
                nc.vector.tensor_sub(
                    lt_sb[:, :cols], lt_sb[:, :cols], lf_sb[:, :cols]
                )
                for c0, w in blocks:
                    nc.tensor.matmul(
                        acc[0:w, 0:w],
                        s_sb[:, c0 : c0 + w],
                        lt_sb[:, c0 : c0 + w],
                        start=(mm == 0),
                        stop=(mm == total_mms - 1),
                    )
                    mm += 1

            mm = 0
            row0 = 0
            pending = []
            for i, r in enumerate(R_TILES):
                cols = r * ATOMS
                a_view = a_dram.ap()[row0 : row0 + P * r, :].rearrange(
                    "(p q) m -> p (q m)", p=P
                )
                f_view = f_dram.ap()[row0 : row0 + P * r, :].rearrange(
                    "(p q) m -> p (q m)", p=P
                )
                row0 += P * r

                a_sb = a_pool.tile([P, R_MAX * ATOMS], mybir.dt.float32)
                f_sb = f_pool.tile([P, R_MAX * ATOMS], mybir.dt.float32)
                nc.sync.dma_start(out=a_sb[:, :cols], in_=a_view)
                # f rides the Activation engine's HWDGE queue so the two
                # input streams use two hardware DMA queues in parallel.
                # The last two tiles' f go on the sync queue instead: the
                # a-stream is done by then, and a trigger stuck behind a
                # tail ACT on the Scalar queue would delay the drain.
                if i >= len(R_TILES) - 1:
                    nc.sync.dma_start(out=f_sb[:, :cols], in_=f_view)
                else:
                    nc.scalar.dma_start(out=f_sb[:, :cols], in_=f_view)

                s_sb = s_pool.tile([P, R_MAX * ATOMS], mybir.dt.bfloat16)
                lt_sb = lt_pool.tile([P, R_MAX * ATOMS], mybir.dt.bfloat16)
                lf_sb = lf_pool.tile([P, R_MAX * ATOMS], mybir.dt.bfloat16)
                tmp = tmp_pool.tile([P, R_MAX], mybir.dt.float32)

                a3 = a_sb[:, :cols].rearrange("p (q m) -> p q m", m=ATOMS)
                s3 = s_sb[:, :cols].rearrange("p (q m) -> p q m", m=ATOMS)
                t1 = tmp[:, 0:r]

                # Boundary columns first (small, keeps the queue ahead of
                # the wide stt that lt waits on): s1 = 3*a0,
                # s50 = a48 + 4*a49 + 4*a50; s0 = 0. All on VectorE so s
                # has a single producer engine (fewer cross-engine waits).
                nc.vector.memset(s3[:, :, 0], 0.0)
                nc.vector.tensor_scalar_mul(s3[:, :, 1], a3[:, :, 0], 3.0)
                nc.vector.scalar_tensor_tensor(
                    out=t1,
                    in0=a3[:, :, 49],
                    scalar=4.0,
                    in1=a3[:, :, 48],
                    op0=mult,
                    op1=add,
                )
                nc.vector.scalar_tensor_tensor(
                    out=s3[:, :, 50],
                    in0=a3[:, :, 50],
                    scalar=4.0,
                    in1=t1,
                    op0=mult,
                    op1=add,
                )
                # s_j = 3*a_{j-1} + a_{j-2} for j in 2..49  (VectorE, wide)
                nc.vector.scalar_tensor_tensor(
                    out=s3[:, :, 2:50],
                    in0=a3[:, :, 1:49],
                    scalar=3.0,
                    in1=a3[:, :, 0:48],
                    op0=mult,
                    op1=add,
                )

                # lf = Ln(4f + 4e-16); lt = Ln(s + 1e-35)  (ScalarE, bf16
                # out). lf first: it depends only on the f DMA, so ScalarE
                # can run it while VectorE is still building s.
                nc.scalar.activation(
                    out=lf_sb[:, :cols],
                    in_=f_sb[:, :cols],
                    func=mybir.ActivationFunctionType.Ln,
                    bias=eps_f[:],
                    scale=4.0,
                )
                nc.scalar.activation(
                    out=lt_sb[:, :cols],
                    in_=s_sb[:, :cols],
                    func=mybir.ActivationFunctionType.Ln,
                    bias=eps_s[:],
                    scale=1.0,
                )
                pending.append((s_sb, lt_sb, lf_sb, cols, tile_blocks[i]))
                if len(pending) > 2:
                    consume(pending.pop(0))

            for p in pending:
                consume(p)

            out_sb = out_pool.tile([P, MM_BLOCK], mybir.dt.float32, tag="out_sb")
            nc.vector.tensor_copy(out_sb[:], acc[:])
            nc.sync.dma_start(out=out_dram.ap(), in_=out_sb[:])

    nc.compile()
    return nc


def kernel(anchor: np.ndarray, feature: np.ndarray) -> np.ndarray:
    global _BUILT, _LAST_RESULTS
    from concourse import bass_utils

    if _BUILT is None:
        _BUILT = _build()
    nc = _BUILT

    anchor = np.ascontiguousarray(anchor, dtype=np.float32)
    feature = np.ascontiguousarray(feature, dtype=np.float32)

    in_maps = []
    for c in range(N_CORES):
        lo, hi = c * ROWS_PER_CORE, (c + 1) * ROWS_PER_CORE
        in_maps.append({"anchor": anchor[lo:hi], "feature": feature[lo:hi]})

    res = bass_utils.run_bass_kernel_spmd(
        nc,
        in_maps,
        core_ids=list(range(N_CORES)),
        trace=bool(os.environ.get("BASS_TRACE")),
    )
    _LAST_RESULTS = res

    total = 0.0
    for c in range(N_CORES):
        total += np.trace(res.results[c]["out"].astype(np.float64))
    val = 0.25 * total / B_TOTAL
    return np.float32(val)
